# revision 35
# baseline (speedup 1.0000x reference)
# Trainium2 Bass kernel for nn_DiffNet.
#
# Math: the conv2(conv1(.)) meta-MLP is affine per element, so with
#   coef = (conv2_w @ conv1_w)[0]  (c0, c1, c2),
#   bc   = (conv2_w @ conv1_b)[0] + conv2_b[0],
#   scale = RATE / batch_num,
# each layer (W, b) of the reference reduces to
#   z  = vi @ W.T                      (pre-bias matmul)
#   vj = relu(z + b)
#   s  = rowsum(vi),  q = rowsum(vi^2)
#   out = (1 + C2*s) * vj + C1*z + (C0*q + Cb*s)
# with C* = scale * (c*, bc).  No [B, out, in] tensor is ever materialized.
#
# Sharding: data-parallel over batch (64 rows -> 8 rows/core), weights
# replicated per core, zero collectives.
#
# Key fusions vs the v0 kernel:
# - alpha*relu(P) + C1*P == PRelu_k(alpha' * P) with alpha' = alpha + C1
#   and per-row slope k = C1/alpha' (alpha' ~ 1 > 0) -> the whole per-layer
#   epilogue is ONE scalar-engine activation (parametric relu with
#   per-partition alpha); its accum_out gives rowsum(o) for free.
# - the per-row constant de2 (delta) is folded into the NEXT layer as a
#   rank-1 update: P_next += de2 (x) colsum(W_next), via a K=2 bias matmul
#   [ones; de2] @ [bhat; wsum].
# - activations are fp16 out of the ACT, transposed on the PE in fp16
#   (PSUM fp16), single copy to SBUF as the next layer's lhsT.
# - the o1 cross-term sum(oc1*(-b1)) is a 4x N=1 PE matvec on the already
#   transposed lhsT chunks instead of a 670ns DVE pass.
# - layer-1 alpha/k/delta depend only on x -> host; the LAST boundary's
#   delta + the final +de2_3+m4 add run on the host too (oc2/oc3/de2_2 are
#   DMA'd out), deleting ~3us of device stat work.
# - w1 is DMA'd in 3 column slices so L1 matmul chunk k starts as soon as
#   its slice lands; the tiny bias-operand pack pk2a goes FIRST on the
#   sync queue (the scalar DMA queue is starved by the wall stream).
#
# Device-side bias folding: P = vi' @ W.T + bhat, where inputs are
# represented as vi = vi' + m (m = -C1*b_prev, m1 = 0) and
# bhat = b + m @ W.T, so P = z + b exactly.
#
# Perf notes (from HW traces):
# - HWDGE descriptor-gen paces a queue at ~650ns/instr; 16 DMA engines
#   stream ~265GB/s aggregate; first packet lands ~1.5us after the trigger.
# - PE HAM clock-gate: warm-up matmuls on junk tiles first, plus pinned
#   keep-warm matmuls after each boundary ACT.
# - Kernel tail pays a fixed ~250-sem reset sequence (~6us) regardless.

import numpy as np

RATE = 0.01
B, IN, H1, H2, OUT = 64, 1024, 512, 512, 256
NCORES = 8
BL = B // NCORES  # 8 rows per core
P128 = 128

# ---- pk8 (fp32, 8 partitions) column map ----
P8_ALPHA1 = 0   # per-row alpha'_1
P8_K1 = 1       # per-row k_1
P8_DE21 = 2     # per-row de2_1
P8_C1 = 3
P8_C0 = 4
P8_CB = 5
P8_C0C1X2 = 6   # 2*C0*C1
P8_C0X2 = 7     # 2*C0
P8_C0N = 8      # C0*512
P8_C2 = 9
P8_C2N = 10     # C2*512
P8_KD = {2: 11, 3: 12}        # kd'_l = C0*sum(m^2)+Cb*sum(m)
P8_CBNCM = {2: 13, 3: 14}     # Cb*n + 2*C0*sum(m)
P8_KA = {2: 15, 3: 16}        # 1 + C2*sum(m) + C1
PK8_LEN = 24

# ---- pk2a (fp16, 2 partitions; FIRST on the sync queue) ----
KA_LHS2 = 0                       # [2,8]: p0 ones, p1 de2_1 (host)
KA_BH1 = 8                        # [1,512] bhat1
KA_RHS2 = KA_BH1 + H1             # [2,512]: p0 bhat2, p1 wsum2
PK2A_LEN = KA_RHS2 + H2
# ---- pk2b (fp16, 8 partitions; scalar queue) ----
KB_RHS3 = 0                       # [2,256]: p0 bhat3, p1 wsum3
KB_ID8 = KB_RHS3 + OUT            # [8,8] identity
PK2B_LEN = KB_ID8 + 8

# ---- wall (fp16): xt | w1 chunks | w2 chunks | w3 chunks ----
XT_OFF = 0
XT_LEN = (IN // P128) * BL  # 64
W_OFF = [XT_LEN, XT_LEN + 4096, XT_LEN + 6144]
NB1C_OFF = XT_LEN + 7168  # [128, 4] chunked -b1 (ce matvec rhs)
W_LEN = NB1C_OFF + 4  # 7236
# DMA split points (sync queue, in priority order)
W_SPLITS = [(0, 1088), (1088, 2624), (2624, 4160), (4160, 6208), (6208, 7236)]

NKS = [IN // P128, H1 // P128, H2 // P128]
NOUTS = [H1, H2, OUT]

N_WARMUP = 6       # PE clock-gate warmup matmuls (N=512)
N_KEEPWARM = 3     # pinned junk matmuls (N=512) after each boundary ACT

_NC_CACHE = {}
DEBUG_TAPS = False


def _build_nc():
    import concourse.bacc as bacc
    import concourse.mybir as mybir
    import concourse.tile as tile
    from concourse.tile_rust import add_dep_helper

    fp32 = mybir.dt.float32
    fp16 = mybir.dt.float16
    AF = mybir.ActivationFunctionType
    ALU = mybir.AluOpType
    AX = mybir.AxisListType

    nc = bacc.Bacc("TRN2", target_bir_lowering=False, debug=False)

    pk8_t = nc.dram_tensor("pk8", [BL, PK8_LEN], fp32, kind="ExternalInput")
    pk2a_t = nc.dram_tensor("pk2a", [2, PK2A_LEN], fp16, kind="ExternalInput")
    pk2b_t = nc.dram_tensor("pk2b", [BL, PK2B_LEN], fp16, kind="ExternalInput")
    w_t = nc.dram_tensor("wall", [P128, W_LEN], fp16, kind="ExternalInput")
    out_t = nc.dram_tensor("outb", [BL, OUT], fp32, kind="ExternalOutput")
    de2_t = nc.dram_tensor("de2b", [BL, 1], fp32, kind="ExternalOutput")
    oc2_t = nc.dram_tensor("oc2b", [BL, H2], fp16, kind="ExternalOutput")

    with tile.TileContext(nc) as tc:
        with (
            tc.tile_pool(name="wp", bufs=1) as wp,
            tc.tile_pool(name="actp", bufs=1) as ap_,
            tc.tile_pool(name="scp", bufs=1) as scp,
            tc.tile_pool(name="pp", bufs=2, space="PSUM") as pp,
            tc.tile_pool(name="tpp", bufs=1, space="PSUM") as tpp,
        ):
            # --- PE warm-up: junk matmuls release the HAM clock gate ---
            junk_a = wp.tile([P128, BL], fp16, tag="junk_a")
            junk_w = wp.tile([P128, 512], fp16, tag="junk_w")
            de2pair = scp.tile([BL, 2], fp16, tag="de2pair")
            nc.gpsimd.memset(junk_a[:], 0.0)
            nc.gpsimd.memset(junk_w[:], 0.0)
            nc.gpsimd.memset(de2pair[:, 0:1], 1.0)
            warm_p = pp.tile([BL, 512], fp32, tag="warm")
            for _ in range(N_WARMUP):
                nc.tensor.matmul(
                    warm_p[:], junk_a[:, :BL], junk_w[:], start=True, stop=True
                )

            # --- DMAs: wall slices on sync in priority order, with the
            # tiny pk2a slotted right after wallA (bias-matmul operands
            # must land deterministically early — the scalar queue gets
            # starved to ~13us once the wall stream saturates the DMA
            # engines — but ahead of wallA it would delay every slice by
            # its ~350ns descriptor-gen time); pk8/pk2b on scalar.
            wseg = []
            pk2a = ap_.tile([2, PK2A_LEN], fp16, tag="pk2a")
            for i, (lo, hi) in enumerate(W_SPLITS):
                t = wp.tile([P128, hi - lo], fp16, tag=f"w{i}")
                nc.sync.dma_start(t[:], w_t[:, lo:hi])
                wseg.append((t, lo))
                if i == 0:
                    nc.sync.dma_start(pk2a[:], pk2a_t[:])
            pk8 = ap_.tile([BL, PK8_LEN], fp32, tag="pk8")
            nc.scalar.dma_start(pk8[:], pk8_t[:])
            pk2b = ap_.tile([BL, PK2B_LEN], fp16, tag="pk2b")
            nc.scalar.dma_start(pk2b[:], pk2b_t[:])

            def wall_slice(lo, n):
                for t, off in wseg:
                    if off <= lo and lo + n <= off + t.shape[1]:
                        return t[:, lo - off : lo - off + n]
                raise AssertionError("bad wall slice")

            def col8(j):
                return pk8[:, j : j + 1]

            id8 = pk2b[:, KB_ID8 : KB_ID8 + 8]

            # L1 lhsT chunks (fp16 [128, BL] each) from the wall
            vt = [[wall_slice(XT_OFF + k * BL, BL) for k in range(NKS[0])]]

            sqjunk = scp.tile([BL, H1], fp16, tag="sqjunk")

            def layer_mms(l, lhs_bias, rhs_bias, bias_first):
                """Accumulate P_l = vi' @ W.T (+ bias rank-1/2) in PSUM.

                For l>0 the bias matmul goes FIRST (start=True): it has no
                boundary-chain dependencies, so P completes at the last
                chunk matmul instead of a trailing bias matmul.  For l=0 it
                goes LAST: the PE queue is in-order and the bias operands
                ride the (slow) scalar DMA queue — putting it first would
                stall every L1 chunk matmul behind that DMA and let the
                HAM clock gate close during the wait.
                """
                nk, nout = NKS[l], NOUTS[l]
                Pt = pp.tile([BL, nout], fp32, tag="P")
                if bias_first:
                    nc.tensor.matmul(
                        Pt[:], lhs_bias, rhs_bias, start=True, stop=False)
                for k in range(nk):
                    nc.tensor.matmul(
                        Pt[:],
                        vt[l][k],
                        wall_slice(W_OFF[l] + k * nout, nout),
                        start=(k == 0 and not bias_first),
                        stop=(k == nk - 1),
                    )
                    if not bias_first and k == nk - 4:
                        # slot the bias matmul into the DMA stall before the
                        # last wall slice: pk2 always lands before wallC, so
                        # it hides there instead of trailing the last chunk
                        nc.tensor.matmul(
                            Pt[:], lhs_bias, rhs_bias, start=False, stop=False)
                return Pt

            def boundary(l, Pt, alpha_ap, k_ap, last=False):
                """LeakyRelu epilogue + fp16 transpose to next lhsT + stats.

                For the last boundary the q/cross statistics are skipped:
                the host recomputes the final delta from the DMA'd oc2.
                """
                nout = NOUTS[l]
                nch = nout // P128
                oc = ap_.tile([BL, nout], fp16, tag=f"oc{l}")
                s_core = scp.tile([BL, 2], fp32, tag=f"sq{l}")
                act = nc.scalar.activation(
                    out=oc[:], in_=Pt[:], func=AF.Prelu,
                    scale=alpha_ap, alpha=k_ap, bias=0.0,
                    accum_out=s_core[:, 0:1],
                )
                # pinned keep-warm: keep the PE HAM clock-gate open through
                # the epilogue gap without delaying next-layer matmuls
                for _ in range(N_KEEPWARM):
                    ji = nc.tensor.matmul(
                        warm_p[:], junk_a[:, :BL], junk_w[:],
                        start=True, stop=True,
                    )
                    add_dep_helper(
                        ji.ins, act.ins, sync=False,
                        reason="pin keep-warm after act",
                    )
                # fp16 PE transposes straight into PSUM (two tiles so the
                # first copy only depends on its own transposes), then one
                # copy per half to SBUF as the next layer's lhsT
                h = nch // 2 if nch > 1 else nch
                tpA = tpp.tile([P128, h * BL], fp16, tag="tpA")
                if nch > h:
                    tpB = tpp.tile([P128, (nch - h) * BL], fp16, tag="tpB")
                else:
                    tpB = None
                for c in range(nch):
                    dst = tpA if c < h else tpB
                    cc = c if c < h else c - h
                    tr_last = nc.tensor.transpose(
                        dst[:, cc * BL : (cc + 1) * BL],
                        oc[:, c * P128 : (c + 1) * P128],
                        id8,
                    )
                if last:
                    for _ in range(3):
                        ji = nc.tensor.matmul(
                            warm_p[:], junk_a[:, :BL], junk_w[:],
                            start=True, stop=True,
                        )
                        add_dep_helper(
                            ji.ins, tr_last.ins, sync=False,
                            reason="bridge keep-warm to L3",
                        )
                vtn = ap_.tile([P128, nch * BL], fp16, tag=f"vt{l + 1}")
                cp0 = nc.vector.tensor_copy(out=vtn[:, : h * BL], in_=tpA[:])
                cp_last = cp0
                if tpB is not None:
                    cp1 = nc.vector.tensor_copy(
                        out=vtn[:, h * BL :], in_=tpB[:]
                    )
                    add_dep_helper(
                        cp1.ins, cp0.ins, sync=False,
                        reason="pin copy-h1 right after copy-h0",
                    )
                    cp_last = cp1
                vt.append([vtn[:, k * BL : (k + 1) * BL] for k in range(nch)])
                if not last:
                    # q = rowsum(oc^2) via ACT accum (cross-term emitted by
                    # the caller, pinned after the alpha chain)
                    nc.scalar.activation(
                        out=sqjunk[:, :nout], in_=oc[:], func=AF.Square,
                        bias=0.0, accum_out=s_core[:, 1:2],
                    )
                return s_core, oc, cp_last, act

            def fixups(l_next, s_core, red_ce, de2_prev, cp_last):
                """alpha'/k (+ optional de2) for layer l_next (DVE).

                Vector-queue order is pinned: lhsT copies first (they gate
                the next layer's matmuls), then the 4-op alpha/k chain
                (gates the next ACT), then the delta chain.  red_ce is the
                PSUM result of the PE cross-term matvec (None: alpha only).
                """
                s_ = s_core[:, 0:1]
                q_ = s_core[:, 1:2]
                a = scp.tile([BL, 2], fp32, tag=f"ak{l_next}")
                a1 = nc.vector.tensor_scalar(
                    a[:, 0:1], s_, col8(P8_C2), col8(P8_KA[l_next]),
                    ALU.mult, ALU.add)
                add_dep_helper(
                    a1.ins, cp_last.ins, sync=False,
                    reason="alpha chain after lhsT copies (vector order)",
                )
                alphan = scp.tile([BL, 1], fp32, tag=f"al{l_next}")
                nc.vector.tensor_scalar(
                    alphan[:], de2_prev, col8(P8_C2N), a[:, 0:1],
                    ALU.mult, ALU.add)
                kn = scp.tile([BL, 1], fp32, tag=f"k{l_next}")
                nc.vector.reciprocal(a[:, 1:2], alphan[:])
                kni = nc.vector.tensor_scalar(
                    kn[:], a[:, 1:2], col8(P8_C1), None, ALU.mult)
                if red_ce is None:
                    return alphan, kn, None
                t = scp.tile([BL, 6], fp32, tag=f"fx{l_next}")
                t1, t2, base, u1, u, v = (t[:, i : i + 1] for i in range(6))
                nc.vector.tensor_scalar(
                    t1, q_, col8(P8_C0), col8(P8_KD[l_next]), ALU.mult, ALU.add)
                nc.vector.tensor_scalar(
                    t2, s_, col8(P8_CB), t1, ALU.mult, ALU.add)
                nc.vector.tensor_scalar(
                    base, red_ce, col8(P8_C0C1X2), t2, ALU.mult, ALU.add)
                nc.vector.tensor_scalar(
                    u1, s_, col8(P8_C0X2), col8(P8_CBNCM[l_next]),
                    ALU.mult, ALU.add)
                nc.vector.tensor_scalar(
                    u, de2_prev, col8(P8_C0N), u1, ALU.mult, ALU.add)
                de2n = scp.tile([BL, 1], fp32, tag=f"de2_{l_next}")
                nc.vector.scalar_tensor_tensor(
                    out=de2n[:], in0=de2_prev, scalar=u, in1=base,
                    op0=ALU.mult, op1=ALU.add)
                return alphan, kn, de2n

            # ---------- layer 1 ----------
            P1 = layer_mms(
                0, pk2a[0:1, KA_LHS2 : KA_LHS2 + BL],
                pk2a[0:1, KA_BH1 : KA_BH1 + H1], bias_first=False,
            )
            s1, oc1, cpl1, act1 = boundary(0, P1, col8(P8_ALPHA1), col8(P8_K1))

            # ---------- layer 2 ----------
            P2 = layer_mms(
                1, pk2a[0:2, KA_LHS2 : KA_LHS2 + BL],
                pk2a[0:2, KA_RHS2 : KA_RHS2 + H2], bias_first=True,
            )

            # cross term ce1 = sum_i oc1*(−b1) as a PE matvec on the
            # already-transposed lhsT chunks (placed after the L2 matmuls
            # in the in-order PE queue so it can't delay them)
            cep = tpp.tile([BL, 1], fp32, tag="cep")
            for k in range(4):
                nc.tensor.matmul(
                    cep[:], vt[1][k],
                    wall_slice(NB1C_OFF + k, 1),
                    start=(k == 0), stop=(k == 3),
                )
            al2, k2, de2_2 = fixups(2, s1, cep[:], col8(P8_DE21), cpl1)

            # de2_2 out to the host (it needs it for the final delta)
            nc.sync.dma_start(de2_t[:], de2_2[:])

            s2, oc2, cpl2, act2 = boundary(1, P2, al2[:], k2[:], last=True)

            # [ones; de2_2] -> fp16 [2,8] lhsT for L3's K=2 bias mm, built
            # via one PE transpose of [8,2] (engines can't address a
            # partition-1 base directly).
            nc.vector.tensor_copy(out=de2pair[:, 1:2], in_=de2_2[:])
            de2T = tpp.tile([2, BL], fp16, tag="de2T")
            nc.tensor.transpose(de2T[:], de2pair[:], id8)
            lhs3 = scp.tile([2, BL], fp16, tag="lhs3")
            nc.vector.tensor_copy(out=lhs3[:], in_=de2T[:])
            al3, k3, _ = fixups(3, s2, None, de2_2[:], cpl2)
            # oc2 out to the host: it recomputes s2/q2/ce2 and the final
            # delta itself, so the whole boundary-2 stat machinery
            # (Square+read-acc+cross+7-op delta chain) never runs on device
            nc.sync.dma_start(oc2_t[:], oc2[:])

            # ---------- layer 3 ----------
            P3 = layer_mms(
                2, lhs3[:],
                pk2b[0:2, KB_RHS3 : KB_RHS3 + OUT], bias_first=True,
            )
            oc3 = ap_.tile([BL, OUT], fp32, tag="oc3")
            act3 = nc.scalar.activation(
                out=oc3[:], in_=P3[:], func=AF.Prelu,
                scale=al3[:], alpha=k3[:], bias=0.0,
            )
            # final out = oc3 + de2_3 + m4 happens on HOST
            nc.sync.dma_start(out_t[:], oc3[:])

            if DEBUG_TAPS:
                for name, ap in (("dbg_o1", oc1[:]), ("dbg_o2", oc2[:])):
                    t = nc.dram_tensor(
                        name, list(ap.shape), ap.dtype, kind="ExternalOutput"
                    )
                    nc.sync.dma_start(t[:], ap)

    nc.compile()
    return nc


def get_nc():
    if "nc" not in _NC_CACHE:
        _NC_CACHE["nc"] = _build_nc()
    return _NC_CACHE["nc"]


def _chunk_pt(a, dtype):
    """[R, C] -> [128, (R//128)*C]: row-chunks of 128 side by side."""
    r, c = a.shape
    nk = r // P128
    return np.ascontiguousarray(
        a.reshape(nk, P128, c).transpose(1, 0, 2).reshape(P128, nk * c), dtype=dtype
    )


def host_prep(x, fc1_w, fc1_b, fc2_w, fc2_b, fc3_w, fc3_b,
              conv1_w, conv1_b, conv2_w, conv2_b, batch_num):
    f32, f16, f64 = np.float32, np.float16, np.float64
    x = np.asarray(x, f32)
    fc1_w = np.asarray(fc1_w, f32)
    fc2_w = np.asarray(fc2_w, f32)
    fc3_w = np.asarray(fc3_w, f32)
    b1 = np.asarray(fc1_b, f64)
    b2 = np.asarray(fc2_b, f64)
    b3 = np.asarray(fc3_b, f64)

    bn = float(np.asarray(batch_num).item())
    scale = RATE / bn
    coef = (np.asarray(conv2_w, f64) @ np.asarray(conv1_w, f64))[0]
    bc = float(
        (np.asarray(conv2_w, f64) @ np.asarray(conv1_b, f64))[0]
        + np.asarray(conv2_b, f64)[0]
    )
    C0, C1, C2 = (scale * coef).astype(f64)
    Cb = scale * bc

    m2 = -C1 * b1
    m3 = -C1 * b2
    m4 = (-C1 * b3).astype(f32)
    bh1 = b1
    bh2 = b2 + m2 @ fc2_w.T.astype(f64)
    bh3 = b3 + m3 @ fc3_w.T.astype(f64)
    wsum2 = fc2_w.astype(f64).sum(axis=1)
    wsum3 = fc3_w.astype(f64).sum(axis=1)

    # ---- pk8 base (constants identical across cores) ----
    pk8_base = np.zeros((BL, PK8_LEN), f32)

    def setc(j, v):
        pk8_base[:, j] = v

    setc(P8_C1, C1)
    setc(P8_C0, C0)
    setc(P8_CB, Cb)
    setc(P8_C0C1X2, 2.0 * C0 * C1)
    setc(P8_C0X2, 2.0 * C0)
    setc(P8_C0N, C0 * 512.0)
    setc(P8_C2, C2)
    setc(P8_C2N, C2 * 512.0)
    for l, m in ((2, m2), (3, m3)):
        setc(P8_KD[l], C0 * float(m @ m) + Cb * float(m.sum()))
        setc(P8_CBNCM[l], Cb * 512.0 + 2.0 * C0 * float(m.sum()))
        setc(P8_KA[l], 1.0 + C2 * float(m.sum()) + C1)

    # ---- pk2a / pk2b base ----
    pk2a_base = np.zeros((2, PK2A_LEN), f16)
    pk2a_base[0, KA_LHS2 : KA_LHS2 + BL] = 1.0
    pk2a_base[0, KA_BH1 : KA_BH1 + H1] = bh1.astype(f16)
    pk2a_base[0, KA_RHS2 : KA_RHS2 + H2] = bh2.astype(f16)
    pk2a_base[1, KA_RHS2 : KA_RHS2 + H2] = wsum2.astype(f16)
    pk2b_base = np.zeros((BL, PK2B_LEN), f16)
    pk2b_base[0, KB_RHS3 : KB_RHS3 + OUT] = bh3.astype(f16)
    pk2b_base[1, KB_RHS3 : KB_RHS3 + OUT] = wsum3.astype(f16)
    pk2b_base[:, KB_ID8 : KB_ID8 + 8] = np.eye(BL, dtype=f16)

    wall_base = np.empty((P128, W_LEN), f16)
    wall_base[:, W_OFF[0] : W_OFF[0] + 4096] = _chunk_pt(fc1_w.T, f16)
    wall_base[:, W_OFF[1] : W_OFF[1] + 2048] = _chunk_pt(fc2_w.T, f16)
    wall_base[:, W_OFF[2] : W_OFF[2] + 1024] = _chunk_pt(fc3_w.T, f16)
    wall_base[:, NB1C_OFF : NB1C_OFF + 4] = _chunk_pt(
        (-b1).reshape(H1, 1), f16)

    post = {
        "m4": m4.astype(f64),
        "nb2": -b2,
        "C0": C0, "Cb": Cb, "C0C1X2": 2.0 * C0 * C1,
        "kd3": C0 * float(m3 @ m3) + Cb * float(m3.sum()),
        "cbncm3": Cb * 512.0 + 2.0 * C0 * float(m3.sum()),
        "C0X2": 2.0 * C0, "C0N": C0 * 512.0,
    }
    in_maps = []
    for c in range(NCORES):
        xk = np.ascontiguousarray(x[c * BL : (c + 1) * BL], dtype=f32)
        sx = xk.astype(f64).sum(axis=1)
        qx = (xk.astype(f64) ** 2).sum(axis=1)
        alpha1 = 1.0 + C2 * sx + C1
        k1 = C1 / alpha1
        de21 = C0 * qx + Cb * sx
        pk8 = pk8_base.copy()
        pk8[:, P8_ALPHA1] = alpha1
        pk8[:, P8_K1] = k1
        pk8[:, P8_DE21] = de21
        pk2a = pk2a_base.copy()
        pk2a[1, KA_LHS2 : KA_LHS2 + BL] = de21.astype(f16)
        wall = wall_base.copy()
        wall[:, XT_OFF : XT_OFF + XT_LEN] = _chunk_pt(xk.T.copy(), f16)
        in_maps.append(
            {"pk8": pk8, "pk2a": pk2a, "pk2b": pk2b_base, "wall": wall})
    return in_maps, post


def finish(res, post):
    """Host-side tail: recompute the final per-row delta from oc2 and
    apply out = oc3 + de2_3 + m4 per core."""
    outs = []
    for c in range(NCORES):
        oc3 = np.asarray(res.results[c]["outb"], np.float64)
        oc2 = np.asarray(res.results[c]["oc2b"], np.float64)
        de2 = np.asarray(res.results[c]["de2b"], np.float64)[:, 0]
        s2 = oc2.sum(axis=1)
        q2 = (oc2 ** 2).sum(axis=1)
        ce2 = (oc2 * post["nb2"][None, :]).sum(axis=1)
        base = (ce2 * post["C0C1X2"]
                + s2 * post["Cb"] + q2 * post["C0"] + post["kd3"])
        u = de2 * post["C0N"] + (s2 * post["C0X2"] + post["cbncm3"])
        de3 = base + de2 * u
        outs.append(oc3 + de3[:, None] + post["m4"][None, :])
    return np.ascontiguousarray(np.concatenate(outs, axis=0), np.float32)


def kernel(**inputs):
    from concourse.bass_utils import run_bass_kernel_spmd

    nc = get_nc()
    in_maps, post = host_prep(**inputs)
    res = run_bass_kernel_spmd(nc, in_maps, core_ids=list(range(NCORES)))
    return finish(res, post)


# revision 36
# speedup vs baseline: 1.0790x; 1.0790x over previous
# Trainium2 Bass kernel for nn_DiffNet.
#
# Math: the conv2(conv1(.)) meta-MLP is affine per element, so with
#   coef = (conv2_w @ conv1_w)[0]  (c0, c1, c2),
#   bc   = (conv2_w @ conv1_b)[0] + conv2_b[0],
#   scale = RATE / batch_num,
# each layer (W, b) of the reference reduces to
#   z  = vi @ W.T                      (pre-bias matmul)
#   vj = relu(z + b)
#   s  = rowsum(vi),  q = rowsum(vi^2)
#   out = (1 + C2*s) * vj + C1*z + (C0*q + Cb*s)
# with C* = scale * (c*, bc).  No [B, out, in] tensor is ever materialized.
#
# Sharding: data-parallel over batch (64 rows -> 8 rows/core), weights
# replicated per core, zero collectives.
#
# Key fusions vs the v0 kernel:
# - alpha*relu(P) + C1*P == PRelu_k(alpha' * P) with alpha' = alpha + C1
#   and per-row slope k = C1/alpha' (alpha' ~ 1 > 0) -> the whole per-layer
#   epilogue is ONE scalar-engine activation (parametric relu with
#   per-partition alpha); its accum_out gives rowsum(o) for free.
# - the per-row constant de2 (delta) is folded into the NEXT layer as a
#   rank-1 update: P_next += de2 (x) colsum(W_next), via a K=2 bias matmul
#   [ones; de2] @ [bhat; wsum].
# - activations are fp16 out of the ACT, transposed on the PE in fp16
#   (PSUM fp16), single copy to SBUF as the next layer's lhsT.
# - the o1 cross-term sum(oc1*(-b1)) is a 4x N=1 PE matvec on the already
#   transposed lhsT chunks instead of a 670ns DVE pass.
# - layer-1 alpha/k/delta depend only on x -> host; the LAST boundary's
#   delta + the final +de2_3+m4 add run on the host too (oc2/oc3/de2_2 are
#   DMA'd out), deleting ~3us of device stat work.
# - w1 is DMA'd in 3 column slices so L1 matmul chunk k starts as soon as
#   its slice lands; the tiny bias-operand pack pk2a goes FIRST on the
#   sync queue (the scalar DMA queue is starved by the wall stream).
#
# Device-side bias folding: P = vi' @ W.T + bhat, where inputs are
# represented as vi = vi' + m (m = -C1*b_prev, m1 = 0) and
# bhat = b + m @ W.T, so P = z + b exactly.
#
# Perf notes (from HW traces):
# - HWDGE descriptor-gen paces a queue at ~650ns/instr; 16 DMA engines
#   stream ~265GB/s aggregate; first packet lands ~1.5us after the trigger.
# - PE HAM clock-gate: warm-up matmuls on junk tiles first, plus pinned
#   keep-warm matmuls after each boundary ACT.
# - Kernel tail pays a fixed ~250-sem reset sequence (~6us) regardless.

import numpy as np

RATE = 0.01
B, IN, H1, H2, OUT = 64, 1024, 512, 512, 256
NCORES = 8
BL = B // NCORES  # 8 rows per core
P128 = 128

# ---- pk8 (fp32, 8 partitions) column map ----
P8_ALPHA1 = 0   # per-row alpha'_1
P8_K1 = 1       # per-row k_1
P8_DE21 = 2     # per-row de2_1
P8_C1 = 3
P8_C0 = 4
P8_CB = 5
P8_C0C1X2 = 6   # 2*C0*C1
P8_C0X2 = 7     # 2*C0
P8_C0N = 8      # C0*512
P8_C2 = 9
P8_C2N = 10     # C2*512
P8_KD = {2: 11, 3: 12}        # kd'_l = C0*sum(m^2)+Cb*sum(m)
P8_CBNCM = {2: 13, 3: 14}     # Cb*n + 2*C0*sum(m)
P8_KA = {2: 15, 3: 16}        # 1 + C2*sum(m) + C1
PK8_LEN = 24

# ---- pk2a (fp16, 2 partitions; FIRST on the sync queue) ----
KA_LHS2 = 0                       # [2,8]: p0 ones, p1 de2_1 (host)
KA_BH1 = 8                        # [1,512] bhat1
KA_RHS2 = KA_BH1 + H1             # [2,512]: p0 bhat2, p1 wsum2
PK2A_LEN = KA_RHS2 + H2
# ---- pk2b (fp16, 8 partitions; scalar queue) ----
KB_RHS3 = 0                       # [2,256]: p0 bhat3, p1 wsum3
KB_ID8 = KB_RHS3 + OUT            # [8,8] identity
PK2B_LEN = KB_ID8 + 8

# ---- wall (fp16): xt | w1 chunks | w2 chunks | w3 chunks ----
XT_OFF = 0
XT_LEN = (IN // P128) * BL  # 64
W_OFF = [XT_LEN, XT_LEN + 4096, XT_LEN + 6144]
NB1C_OFF = XT_LEN + 7168  # [128, 4] chunked -b1 (ce matvec rhs)
W_LEN = NB1C_OFF + 4  # 7236
# DMA split points (sync queue, in priority order).  Finer w1 slices:
# the completion semaphore fires up to ~1.3us after a slice's last data
# packet, so small final slices unblock their matmuls sooner.
W_SPLITS = [(0, 1088), (1088, 2112), (2112, 2624), (2624, 3648),
            (3648, 4160), (4160, 6208), (6208, 7236)]

NKS = [IN // P128, H1 // P128, H2 // P128]
NOUTS = [H1, H2, OUT]

N_WARMUP = 6       # PE clock-gate warmup matmuls (N=512)
N_KEEPWARM = 3     # pinned junk matmuls (N=512) after each boundary ACT

_NC_CACHE = {}
DEBUG_TAPS = False


def _build_nc():
    import concourse.bacc as bacc
    import concourse.mybir as mybir
    import concourse.tile as tile
    from concourse.tile_rust import add_dep_helper

    fp32 = mybir.dt.float32
    fp16 = mybir.dt.float16
    AF = mybir.ActivationFunctionType
    ALU = mybir.AluOpType
    AX = mybir.AxisListType

    nc = bacc.Bacc("TRN2", target_bir_lowering=False, debug=False)

    pk8_t = nc.dram_tensor("pk8", [BL, PK8_LEN], fp32, kind="ExternalInput")
    pk2a_t = nc.dram_tensor("pk2a", [2, PK2A_LEN], fp16, kind="ExternalInput")
    pk2b_t = nc.dram_tensor("pk2b", [BL, PK2B_LEN], fp16, kind="ExternalInput")
    w_t = nc.dram_tensor("wall", [P128, W_LEN], fp16, kind="ExternalInput")
    out_t = nc.dram_tensor("outb", [BL, OUT], fp32, kind="ExternalOutput")
    de2_t = nc.dram_tensor("de2b", [BL, 1], fp32, kind="ExternalOutput")
    oc2_t = nc.dram_tensor("oc2b", [BL, H2], fp16, kind="ExternalOutput")

    with tile.TileContext(nc) as tc:
        with (
            tc.tile_pool(name="wp", bufs=1) as wp,
            tc.tile_pool(name="actp", bufs=1) as ap_,
            tc.tile_pool(name="scp", bufs=1) as scp,
            tc.tile_pool(name="pp", bufs=2, space="PSUM") as pp,
            tc.tile_pool(name="tpp", bufs=1, space="PSUM") as tpp,
        ):
            # --- PE warm-up: junk matmuls release the HAM clock gate ---
            junk_a = wp.tile([P128, BL], fp16, tag="junk_a")
            junk_w = wp.tile([P128, 512], fp16, tag="junk_w")
            de2pair = scp.tile([BL, 2], fp16, tag="de2pair")
            nc.gpsimd.memset(junk_a[:], 0.0)
            nc.gpsimd.memset(junk_w[:], 0.0)
            nc.gpsimd.memset(de2pair[:, 0:1], 1.0)
            warm_p = pp.tile([BL, 512], fp32, tag="warm")
            for _ in range(N_WARMUP):
                nc.tensor.matmul(
                    warm_p[:], junk_a[:, :BL], junk_w[:], start=True, stop=True
                )

            # --- DMAs: wall slices on sync in priority order, with the
            # tiny pk2a slotted right after wallA (bias-matmul operands
            # must land deterministically early — the scalar queue gets
            # starved to ~13us once the wall stream saturates the DMA
            # engines — but ahead of wallA it would delay every slice by
            # its ~350ns descriptor-gen time); pk8/pk2b on scalar.
            wseg = []
            pk2a = ap_.tile([2, PK2A_LEN], fp16, tag="pk2a")
            for i, (lo, hi) in enumerate(W_SPLITS):
                t = wp.tile([P128, hi - lo], fp16, tag=f"w{i}")
                nc.sync.dma_start(t[:], w_t[:, lo:hi])
                wseg.append((t, lo))
                if i == 0:
                    nc.sync.dma_start(pk2a[:], pk2a_t[:])
            pk8 = ap_.tile([BL, PK8_LEN], fp32, tag="pk8")
            nc.scalar.dma_start(pk8[:], pk8_t[:])
            pk2b = ap_.tile([BL, PK2B_LEN], fp16, tag="pk2b")
            nc.scalar.dma_start(pk2b[:], pk2b_t[:])

            def wall_slice(lo, n):
                for t, off in wseg:
                    if off <= lo and lo + n <= off + t.shape[1]:
                        return t[:, lo - off : lo - off + n]
                raise AssertionError("bad wall slice")

            def col8(j):
                return pk8[:, j : j + 1]

            id8 = pk2b[:, KB_ID8 : KB_ID8 + 8]

            # L1 lhsT chunks (fp16 [128, BL] each) from the wall
            vt = [[wall_slice(XT_OFF + k * BL, BL) for k in range(NKS[0])]]

            sqjunk = scp.tile([BL, H1], fp16, tag="sqjunk")

            def layer_mms(l, lhs_bias, rhs_bias, bias_first):
                """Accumulate P_l = vi' @ W.T (+ bias rank-1/2) in PSUM.

                For l>0 the bias matmul goes FIRST (start=True): it has no
                boundary-chain dependencies, so P completes at the last
                chunk matmul instead of a trailing bias matmul.  For l=0 it
                goes LAST: the PE queue is in-order and the bias operands
                ride the (slow) scalar DMA queue — putting it first would
                stall every L1 chunk matmul behind that DMA and let the
                HAM clock gate close during the wait.
                """
                nk, nout = NKS[l], NOUTS[l]
                Pt = pp.tile([BL, nout], fp32, tag="P")
                if bias_first:
                    nc.tensor.matmul(
                        Pt[:], lhs_bias, rhs_bias, start=True, stop=False)
                for k in range(nk):
                    nc.tensor.matmul(
                        Pt[:],
                        vt[l][k],
                        wall_slice(W_OFF[l] + k * nout, nout),
                        start=(k == 0 and not bias_first),
                        stop=(k == nk - 1),
                    )
                    if not bias_first and k == nk - 4:
                        # slot the bias matmul + keep-warm junk into the DMA
                        # stall before the later wall slices: they hide in
                        # the stall and keep the HAM clock gate open so the
                        # post-stall matmuls run at full rate
                        nc.tensor.matmul(
                            Pt[:], lhs_bias, rhs_bias, start=False, stop=False)
                        for _ in range(2):
                            nc.tensor.matmul(
                                warm_p[:], junk_a[:, :BL], junk_w[:],
                                start=True, stop=True)
                return Pt

            def boundary(l, Pt, alpha_ap, k_ap, last=False):
                """LeakyRelu epilogue + fp16 transpose to next lhsT + stats.

                For the last boundary the q/cross statistics are skipped:
                the host recomputes the final delta from the DMA'd oc2.
                """
                nout = NOUTS[l]
                nch = nout // P128
                oc = ap_.tile([BL, nout], fp16, tag=f"oc{l}")
                s_core = scp.tile([BL, 2], fp32, tag=f"sq{l}")
                act = nc.scalar.activation(
                    out=oc[:], in_=Pt[:], func=AF.Prelu,
                    scale=alpha_ap, alpha=k_ap, bias=0.0,
                    accum_out=s_core[:, 0:1],
                )
                # pinned keep-warm: keep the PE HAM clock-gate open through
                # the epilogue gap without delaying next-layer matmuls
                for _ in range(N_KEEPWARM):
                    ji = nc.tensor.matmul(
                        warm_p[:], junk_a[:, :BL], junk_w[:],
                        start=True, stop=True,
                    )
                    add_dep_helper(
                        ji.ins, act.ins, sync=False,
                        reason="pin keep-warm after act",
                    )
                # fp16 PE transposes straight into PSUM (two tiles so the
                # first copy only depends on its own transposes), then one
                # copy per half to SBUF as the next layer's lhsT
                h = nch // 2 if nch > 1 else nch
                tpA = tpp.tile([P128, h * BL], fp16, tag="tpA")
                if nch > h:
                    tpB = tpp.tile([P128, (nch - h) * BL], fp16, tag="tpB")
                else:
                    tpB = None
                for c in range(nch):
                    dst = tpA if c < h else tpB
                    cc = c if c < h else c - h
                    tr_last = nc.tensor.transpose(
                        dst[:, cc * BL : (cc + 1) * BL],
                        oc[:, c * P128 : (c + 1) * P128],
                        id8,
                    )
                if last:
                    for _ in range(3):
                        ji = nc.tensor.matmul(
                            warm_p[:], junk_a[:, :BL], junk_w[:],
                            start=True, stop=True,
                        )
                        add_dep_helper(
                            ji.ins, tr_last.ins, sync=False,
                            reason="bridge keep-warm to L3",
                        )
                vtn = ap_.tile([P128, nch * BL], fp16, tag=f"vt{l + 1}")
                cp0 = nc.vector.tensor_copy(out=vtn[:, : h * BL], in_=tpA[:])
                cp_last = cp0
                if tpB is not None:
                    cp1 = nc.vector.tensor_copy(
                        out=vtn[:, h * BL :], in_=tpB[:]
                    )
                    add_dep_helper(
                        cp1.ins, cp0.ins, sync=False,
                        reason="pin copy-h1 right after copy-h0",
                    )
                    cp_last = cp1
                vt.append([vtn[:, k * BL : (k + 1) * BL] for k in range(nch)])
                if not last:
                    # q = rowsum(oc^2) via ACT accum (cross-term emitted by
                    # the caller, pinned after the alpha chain)
                    nc.scalar.activation(
                        out=sqjunk[:, :nout], in_=oc[:], func=AF.Square,
                        bias=0.0, accum_out=s_core[:, 1:2],
                    )
                return s_core, oc, cp_last, act

            def fixups(l_next, s_core, red_ce, de2_prev, cp_last):
                """alpha'/k (+ optional de2) for layer l_next (DVE).

                Vector-queue order is pinned: lhsT copies first (they gate
                the next layer's matmuls), then the 4-op alpha/k chain
                (gates the next ACT), then the delta chain.  red_ce is the
                PSUM result of the PE cross-term matvec (None: alpha only).
                """
                s_ = s_core[:, 0:1]
                q_ = s_core[:, 1:2]
                a = scp.tile([BL, 2], fp32, tag=f"ak{l_next}")
                a1 = nc.vector.tensor_scalar(
                    a[:, 0:1], s_, col8(P8_C2), col8(P8_KA[l_next]),
                    ALU.mult, ALU.add)
                add_dep_helper(
                    a1.ins, cp_last.ins, sync=False,
                    reason="alpha chain after lhsT copies (vector order)",
                )
                alphan = scp.tile([BL, 1], fp32, tag=f"al{l_next}")
                nc.vector.tensor_scalar(
                    alphan[:], de2_prev, col8(P8_C2N), a[:, 0:1],
                    ALU.mult, ALU.add)
                kn = scp.tile([BL, 1], fp32, tag=f"k{l_next}")
                nc.vector.reciprocal(a[:, 1:2], alphan[:])
                kni = nc.vector.tensor_scalar(
                    kn[:], a[:, 1:2], col8(P8_C1), None, ALU.mult)
                if red_ce is None:
                    return alphan, kn, None
                t = scp.tile([BL, 6], fp32, tag=f"fx{l_next}")
                t1, t2, base, u1, u, v = (t[:, i : i + 1] for i in range(6))
                nc.vector.tensor_scalar(
                    t1, q_, col8(P8_C0), col8(P8_KD[l_next]), ALU.mult, ALU.add)
                nc.vector.tensor_scalar(
                    t2, s_, col8(P8_CB), t1, ALU.mult, ALU.add)
                nc.vector.tensor_scalar(
                    base, red_ce, col8(P8_C0C1X2), t2, ALU.mult, ALU.add)
                nc.vector.tensor_scalar(
                    u1, s_, col8(P8_C0X2), col8(P8_CBNCM[l_next]),
                    ALU.mult, ALU.add)
                nc.vector.tensor_scalar(
                    u, de2_prev, col8(P8_C0N), u1, ALU.mult, ALU.add)
                de2n = scp.tile([BL, 1], fp32, tag=f"de2_{l_next}")
                nc.vector.scalar_tensor_tensor(
                    out=de2n[:], in0=de2_prev, scalar=u, in1=base,
                    op0=ALU.mult, op1=ALU.add)
                return alphan, kn, de2n

            # ---------- layer 1 ----------
            P1 = layer_mms(
                0, pk2a[0:1, KA_LHS2 : KA_LHS2 + BL],
                pk2a[0:1, KA_BH1 : KA_BH1 + H1], bias_first=False,
            )
            s1, oc1, cpl1, act1 = boundary(0, P1, col8(P8_ALPHA1), col8(P8_K1))

            # ---------- layer 2 ----------
            P2 = layer_mms(
                1, pk2a[0:2, KA_LHS2 : KA_LHS2 + BL],
                pk2a[0:2, KA_RHS2 : KA_RHS2 + H2], bias_first=True,
            )

            # cross term ce1 = sum_i oc1*(−b1) as a PE matvec on the
            # already-transposed lhsT chunks (placed after the L2 matmuls
            # in the in-order PE queue so it can't delay them)
            cep = tpp.tile([BL, 1], fp32, tag="cep")
            for k in range(4):
                nc.tensor.matmul(
                    cep[:], vt[1][k],
                    wall_slice(NB1C_OFF + k, 1),
                    start=(k == 0), stop=(k == 3),
                )
            al2, k2, de2_2 = fixups(2, s1, cep[:], col8(P8_DE21), cpl1)

            # de2_2 out to the host (it needs it for the final delta)
            nc.sync.dma_start(de2_t[:], de2_2[:])

            s2, oc2, cpl2, act2 = boundary(1, P2, al2[:], k2[:], last=True)

            # [ones; de2_2] -> fp16 [2,8] lhsT for L3's K=2 bias mm, built
            # via one PE transpose of [8,2] (engines can't address a
            # partition-1 base directly).
            nc.vector.tensor_copy(out=de2pair[:, 1:2], in_=de2_2[:])
            de2T = tpp.tile([2, BL], fp16, tag="de2T")
            nc.tensor.transpose(de2T[:], de2pair[:], id8)
            lhs3 = scp.tile([2, BL], fp16, tag="lhs3")
            nc.vector.tensor_copy(out=lhs3[:], in_=de2T[:])
            al3, k3, _ = fixups(3, s2, None, de2_2[:], cpl2)
            # oc2 out to the host: it recomputes s2/q2/ce2 and the final
            # delta itself, so the whole boundary-2 stat machinery
            # (Square+read-acc+cross+7-op delta chain) never runs on device
            nc.sync.dma_start(oc2_t[:], oc2[:])

            # ---------- layer 3 ----------
            P3 = layer_mms(
                2, lhs3[:],
                pk2b[0:2, KB_RHS3 : KB_RHS3 + OUT], bias_first=True,
            )
            oc3 = ap_.tile([BL, OUT], fp32, tag="oc3")
            act3 = nc.scalar.activation(
                out=oc3[:], in_=P3[:], func=AF.Prelu,
                scale=al3[:], alpha=k3[:], bias=0.0,
            )
            # final out = oc3 + de2_3 + m4 happens on HOST
            nc.sync.dma_start(out_t[:], oc3[:])

            if DEBUG_TAPS:
                for name, ap in (("dbg_o1", oc1[:]), ("dbg_o2", oc2[:])):
                    t = nc.dram_tensor(
                        name, list(ap.shape), ap.dtype, kind="ExternalOutput"
                    )
                    nc.sync.dma_start(t[:], ap)

    nc.compile()
    return nc


def get_nc():
    if "nc" not in _NC_CACHE:
        _NC_CACHE["nc"] = _build_nc()
    return _NC_CACHE["nc"]


def _chunk_pt(a, dtype):
    """[R, C] -> [128, (R//128)*C]: row-chunks of 128 side by side."""
    r, c = a.shape
    nk = r // P128
    return np.ascontiguousarray(
        a.reshape(nk, P128, c).transpose(1, 0, 2).reshape(P128, nk * c), dtype=dtype
    )


def host_prep(x, fc1_w, fc1_b, fc2_w, fc2_b, fc3_w, fc3_b,
              conv1_w, conv1_b, conv2_w, conv2_b, batch_num):
    f32, f16, f64 = np.float32, np.float16, np.float64
    x = np.asarray(x, f32)
    fc1_w = np.asarray(fc1_w, f32)
    fc2_w = np.asarray(fc2_w, f32)
    fc3_w = np.asarray(fc3_w, f32)
    b1 = np.asarray(fc1_b, f64)
    b2 = np.asarray(fc2_b, f64)
    b3 = np.asarray(fc3_b, f64)

    bn = float(np.asarray(batch_num).item())
    scale = RATE / bn
    coef = (np.asarray(conv2_w, f64) @ np.asarray(conv1_w, f64))[0]
    bc = float(
        (np.asarray(conv2_w, f64) @ np.asarray(conv1_b, f64))[0]
        + np.asarray(conv2_b, f64)[0]
    )
    C0, C1, C2 = (scale * coef).astype(f64)
    Cb = scale * bc

    m2 = -C1 * b1
    m3 = -C1 * b2
    m4 = (-C1 * b3).astype(f32)
    bh1 = b1
    bh2 = b2 + m2 @ fc2_w.T.astype(f64)
    bh3 = b3 + m3 @ fc3_w.T.astype(f64)
    wsum2 = fc2_w.astype(f64).sum(axis=1)
    wsum3 = fc3_w.astype(f64).sum(axis=1)

    # ---- pk8 base (constants identical across cores) ----
    pk8_base = np.zeros((BL, PK8_LEN), f32)

    def setc(j, v):
        pk8_base[:, j] = v

    setc(P8_C1, C1)
    setc(P8_C0, C0)
    setc(P8_CB, Cb)
    setc(P8_C0C1X2, 2.0 * C0 * C1)
    setc(P8_C0X2, 2.0 * C0)
    setc(P8_C0N, C0 * 512.0)
    setc(P8_C2, C2)
    setc(P8_C2N, C2 * 512.0)
    for l, m in ((2, m2), (3, m3)):
        setc(P8_KD[l], C0 * float(m @ m) + Cb * float(m.sum()))
        setc(P8_CBNCM[l], Cb * 512.0 + 2.0 * C0 * float(m.sum()))
        setc(P8_KA[l], 1.0 + C2 * float(m.sum()) + C1)

    # ---- pk2a / pk2b base ----
    pk2a_base = np.zeros((2, PK2A_LEN), f16)
    pk2a_base[0, KA_LHS2 : KA_LHS2 + BL] = 1.0
    pk2a_base[0, KA_BH1 : KA_BH1 + H1] = bh1.astype(f16)
    pk2a_base[0, KA_RHS2 : KA_RHS2 + H2] = bh2.astype(f16)
    pk2a_base[1, KA_RHS2 : KA_RHS2 + H2] = wsum2.astype(f16)
    pk2b_base = np.zeros((BL, PK2B_LEN), f16)
    pk2b_base[0, KB_RHS3 : KB_RHS3 + OUT] = bh3.astype(f16)
    pk2b_base[1, KB_RHS3 : KB_RHS3 + OUT] = wsum3.astype(f16)
    pk2b_base[:, KB_ID8 : KB_ID8 + 8] = np.eye(BL, dtype=f16)

    wall_base = np.empty((P128, W_LEN), f16)
    wall_base[:, W_OFF[0] : W_OFF[0] + 4096] = _chunk_pt(fc1_w.T, f16)
    wall_base[:, W_OFF[1] : W_OFF[1] + 2048] = _chunk_pt(fc2_w.T, f16)
    wall_base[:, W_OFF[2] : W_OFF[2] + 1024] = _chunk_pt(fc3_w.T, f16)
    wall_base[:, NB1C_OFF : NB1C_OFF + 4] = _chunk_pt(
        (-b1).reshape(H1, 1), f16)

    post = {
        "m4": m4.astype(f64),
        "nb2": -b2,
        "C0": C0, "Cb": Cb, "C0C1X2": 2.0 * C0 * C1,
        "kd3": C0 * float(m3 @ m3) + Cb * float(m3.sum()),
        "cbncm3": Cb * 512.0 + 2.0 * C0 * float(m3.sum()),
        "C0X2": 2.0 * C0, "C0N": C0 * 512.0,
    }
    in_maps = []
    for c in range(NCORES):
        xk = np.ascontiguousarray(x[c * BL : (c + 1) * BL], dtype=f32)
        sx = xk.astype(f64).sum(axis=1)
        qx = (xk.astype(f64) ** 2).sum(axis=1)
        alpha1 = 1.0 + C2 * sx + C1
        k1 = C1 / alpha1
        de21 = C0 * qx + Cb * sx
        pk8 = pk8_base.copy()
        pk8[:, P8_ALPHA1] = alpha1
        pk8[:, P8_K1] = k1
        pk8[:, P8_DE21] = de21
        pk2a = pk2a_base.copy()
        pk2a[1, KA_LHS2 : KA_LHS2 + BL] = de21.astype(f16)
        wall = wall_base.copy()
        wall[:, XT_OFF : XT_OFF + XT_LEN] = _chunk_pt(xk.T.copy(), f16)
        in_maps.append(
            {"pk8": pk8, "pk2a": pk2a, "pk2b": pk2b_base, "wall": wall})
    return in_maps, post


def finish(res, post):
    """Host-side tail: recompute the final per-row delta from oc2 and
    apply out = oc3 + de2_3 + m4 per core."""
    outs = []
    for c in range(NCORES):
        oc3 = np.asarray(res.results[c]["outb"], np.float64)
        oc2 = np.asarray(res.results[c]["oc2b"], np.float64)
        de2 = np.asarray(res.results[c]["de2b"], np.float64)[:, 0]
        s2 = oc2.sum(axis=1)
        q2 = (oc2 ** 2).sum(axis=1)
        ce2 = (oc2 * post["nb2"][None, :]).sum(axis=1)
        base = (ce2 * post["C0C1X2"]
                + s2 * post["Cb"] + q2 * post["C0"] + post["kd3"])
        u = de2 * post["C0N"] + (s2 * post["C0X2"] + post["cbncm3"])
        de3 = base + de2 * u
        outs.append(oc3 + de3[:, None] + post["m4"][None, :])
    return np.ascontiguousarray(np.concatenate(outs, axis=0), np.float32)


def kernel(**inputs):
    from concourse.bass_utils import run_bass_kernel_spmd

    nc = get_nc()
    in_maps, post = host_prep(**inputs)
    res = run_bass_kernel_spmd(nc, in_maps, core_ids=list(range(NCORES)))
    return finish(res, post)
